# revision 13
# baseline (speedup 1.0000x reference)
"""CenterLoss Trainium2 kernel.

loss = mean_i ||x[i] - centers[labels[i]]||^2

The one-hot-masked distance matrix in the reference collapses to a row
gather of `centers`, so the kernel is a gather + fused square-reduce:
data-parallel over 8 NeuronCores (512 batch rows each, centers
replicated), with the final 8-way scalar all-reduce done on host.

The gather uses the Q7 dma_gather custom op (one instruction per 512
rows) split into two calls because its indices are int16: call 1
gathers rows with label < 32768, call 2 gathers label >= 32768 against
a centers[32768:] base; out-of-range slots are encoded as -1, which
the ucode skips in place.
"""

import os
import sys

import numpy as np

for _p in ("/opt/trn_rl_repo", "/root/.axon_site/_ro/trn_rl_repo", "/root/.axon_site", "/root/.axon_site/_ro/pypackages"):
    if os.path.isdir(_p) and _p not in sys.path:
        sys.path.append(_p)

NCORES = 8
B = 4096
D = 128
C = 50000
P = 128
B_LOC = B // NCORES          # 512 rows per core
NTILES = B_LOC // P          # 4 row-tiles of 128
HALF = 32768                 # int16 index range split

VARIANT = os.environ.get("CL_VARIANT", "dma_gather")

_cached = {}


def _build(variant):
    import concourse.bacc as bacc
    import concourse.bass as bass
    import concourse.mybir as mybir
    import concourse.tile as tile

    nc = bacc.Bacc(
        "TRN2",
        target_bir_lowering=False,
        debug=False,
        enable_asserts=False,
        num_devices=NCORES,
    )
    x = nc.dram_tensor("x", [B_LOC, D], mybir.dt.float32, kind="ExternalInput").ap()
    centers = nc.dram_tensor("centers", [C, D], mybir.dt.float32, kind="ExternalInput").ap()
    if variant == "indirect_acc":
        out = nc.dram_tensor("out", [P, NTILES], mybir.dt.float32, kind="ExternalOutput").ap()
    else:
        out = nc.dram_tensor("out", [1, 1], mybir.dt.float32, kind="ExternalOutput").ap()

    # x[n*P + p, d] -> partition p, free column n*D + d
    x_src = x.rearrange("(n p) d -> p n d", p=P)

    if variant == "dma_gather":
        # [128, 64] int16: cols 0..31 = low-range pack, 32..63 = high-range pack
        labels = nc.dram_tensor("labels", [P, 2 * B_LOC // 16], mybir.dt.int16, kind="ExternalInput").ap()
    else:
        labels = nc.dram_tensor("labels", [P, NTILES], mybir.dt.int32, kind="ExternalInput").ap()

    with tile.TileContext(nc) as tc:
        with (
            tc.tile_pool(name="sbuf", bufs=1) as pool,
            tc.tile_pool(name="psum", bufs=1, space="PSUM") as psum_pool,
        ):
            x_all = pool.tile([P, NTILES * D], mybir.dt.float32)
            ones = pool.tile([P, 1], mybir.dt.float32)
            res = pool.tile([1, 1], mybir.dt.float32)

            nc.vector.memset(ones[:], 1.0)

            if variant == "dma_gather":
                idx_all = pool.tile([P, 2 * B_LOC // 16], mybir.dt.int16)
                g = pool.tile([P, NTILES * D], mybir.dt.float32)
                acc = pool.tile([P, 1], mybir.dt.float32)

                nc.sync.dma_start(out=idx_all[:], in_=labels[:])
                nc.scalar.dma_start(out=x_all[:].rearrange("p (n d) -> p n d", d=D), in_=x_src)
                g_view = g[:].rearrange("p (n d) -> p n d", d=D)
                nc.gpsimd.dma_gather(
                    out_ap=g_view, in_ap=centers[:],
                    idxs_ap=idx_all[:, : B_LOC // 16],
                    num_idxs=B_LOC, num_idxs_reg=B_LOC, elem_size=D,
                )
                nc.gpsimd.dma_gather(
                    out_ap=g_view, in_ap=centers[HALF:, :],
                    idxs_ap=idx_all[:, B_LOC // 16 :],
                    num_idxs=B_LOC, num_idxs_reg=B_LOC, elem_size=D,
                )
                nc.vector.tensor_tensor(
                    out=g[:], in0=x_all[:], in1=g[:], op=mybir.AluOpType.subtract
                )
                nc.scalar.activation(
                    out=g[:], in_=g[:],
                    func=mybir.ActivationFunctionType.Square,
                    accum_out=acc[:],
                )
                ps = psum_pool.tile([1, 1], mybir.dt.float32)
                nc.tensor.matmul(out=ps[:], lhsT=acc[:], rhs=ones[:], start=True, stop=True)
                nc.scalar.copy(res[:], ps[:])
                nc.sync.dma_start(out=out[:], in_=res[:])
            else:
                idx_all = pool.tile([P, NTILES], mybir.dt.int32)
                acc = pool.tile([P, NTILES], mybir.dt.float32)
                c_t = [pool.tile([P, D], mybir.dt.float32, name=f"c{i}") for i in range(NTILES)]

                nc.sync.dma_start(out=idx_all[:], in_=labels[:])
                nc.scalar.dma_start(out=x_all[:].rearrange("p (n d) -> p n d", d=D), in_=x_src)
                if os.environ.get("CL_CRITICAL", "0") == "1":
                    with tc.tile_critical():
                        for i in range(NTILES):
                            nc.gpsimd.indirect_dma_start(
                                out=c_t[i][:],
                                out_offset=None,
                                in_=centers[:],
                                in_offset=bass.IndirectOffsetOnAxis(ap=idx_all[:, i : i + 1], axis=0),
                            )
                else:
                    for i in range(NTILES):
                        nc.gpsimd.indirect_dma_start(
                            out=c_t[i][:],
                            out_offset=None,
                            in_=centers[:],
                            in_offset=bass.IndirectOffsetOnAxis(ap=idx_all[:, i : i + 1], axis=0),
                        )
                use_act = os.environ.get("CL_ACT", "0") == "1"
                for i in range(NTILES):
                    nc.vector.tensor_tensor(
                        out=c_t[i][:],
                        in0=x_all[:, i * D : (i + 1) * D],
                        in1=c_t[i][:],
                        op=mybir.AluOpType.subtract,
                    )
                    if use_act:
                        nc.scalar.activation(
                            out=c_t[i][:],
                            in_=c_t[i][:],
                            func=mybir.ActivationFunctionType.Square,
                            accum_out=acc[:, i : i + 1],
                        )
                    else:
                        nc.vector.tensor_tensor(
                            out=c_t[i][:],
                            in0=c_t[i][:],
                            in1=c_t[i][:],
                            op=mybir.AluOpType.mult,
                        )
                        nc.vector.tensor_reduce(
                            out=acc[:, i : i + 1],
                            in_=c_t[i][:],
                            axis=mybir.AxisListType.X,
                            op=mybir.AluOpType.add,
                        )
                if variant == "indirect_acc":
                    # ship per-partition partial sums; host finishes the reduce
                    nc.sync.dma_start(out=out[:], in_=acc[:])
                else:
                    col = pool.tile([P, 1], mybir.dt.float32)
                    nc.vector.tensor_reduce(
                        out=col[:], in_=acc[:],
                        axis=mybir.AxisListType.X, op=mybir.AluOpType.add,
                    )
                    ps = psum_pool.tile([1, 1], mybir.dt.float32)
                    nc.tensor.matmul(out=ps[:], lhsT=col[:], rhs=ones[:], start=True, stop=True)
                    nc.scalar.copy(res[:], ps[:])
                    nc.sync.dma_start(out=out[:], in_=res[:])

    nc.compile()
    return nc


def _get_nc(variant):
    if variant not in _cached:
        _cached[variant] = _build(variant)
    return _cached[variant]


def _pack16(arr):
    """dma_gather index layout: logical index k read from [k % 16, k // 16],
    replicated across the 8 16-partition groups."""
    w = arr.reshape(-1, 16).T.astype(np.int16)      # [16, n/16]
    return np.tile(w, (8, 1))                        # [128, n/16]


def kernel(x, labels, centers, **profile_kwargs):
    from concourse.bass_utils import run_bass_kernel_spmd

    variant = VARIANT
    nc = _get_nc(variant)
    x = np.ascontiguousarray(np.asarray(x), dtype=np.float32)
    centers = np.ascontiguousarray(np.asarray(centers), dtype=np.float32)
    labels64 = np.asarray(labels).astype(np.int64)

    in_maps = []
    for k in range(NCORES):
        xs = x[k * B_LOC : (k + 1) * B_LOC]
        lab = labels64[k * B_LOC : (k + 1) * B_LOC]
        if variant == "dma_gather":
            lo = np.where(lab < HALF, lab, -1).astype(np.int16)
            hi = np.where(lab >= HALF, lab - HALF, -1).astype(np.int16)
            ls = np.ascontiguousarray(
                np.concatenate([_pack16(lo), _pack16(hi)], axis=1)
            )
        else:
            ls = np.ascontiguousarray(
                lab.astype(np.int32).reshape(NTILES, P).T
            )
        in_maps.append({"x": xs, "labels": ls, "centers": centers})

    r = run_bass_kernel_spmd(nc, in_maps, core_ids=list(range(NCORES)), **profile_kwargs)
    # per-core outputs are raw (partial) sums of squared distances; the mean
    # is the host-side all-reduce
    total = sum(float(m["out"].sum()) for m in r.results)
    out = np.array(total / B, dtype=np.float32)
    if profile_kwargs:
        return out, r
    return out
